# revision 1
# baseline (speedup 1.0000x reference)
"""Cross-attention kernel for 8 Trainium2 NeuronCores.

Sharding: core c => batch b = c//4, head-group g = c%4 (3 of 12 heads, 192 dims).
Each core projects q/k/v for its heads, does softmax attention, and computes a
partial output projection (row-split Wo); host sums the 4 partials per batch.

Key tricks:
  - mask compaction on host: only mask==1 key/value positions are shipped
    (~2048 of 4096), zero-padded to a multiple of 128. Padded rows have
    zeroed v and zeroed ones-column so they contribute 0 to both numerator
    and denominator => exact equivalence with the reference's -1e4 bias.
  - transposed layouts end to end (contraction dim on partitions): no
    on-device transposes.
  - softmax without max-subtraction (scores*scale ~ N(0,1): exp safe in
    fp32) and without dividing the SxN score matrix: a ones-column appended
    to v yields the denominator Z per output row; only the 64xN attention
    output is normalized.
  - fp16 operands for all matmuls (fp32 PSUM accumulate).
  - q/k/v projections are spread through the attention j-loops as PE filler
    so the PE stream stays dense (HAM clock gate at 8/8), with emission
    software-pipelined (scores j+1 issued before attn j).
"""

import numpy as np

import concourse.bass as bass
import concourse.mybir as mybir
import concourse.tile as tile
from concourse import bacc
from concourse.bass_utils import run_bass_kernel_spmd

H = 12
D = 768
HD = 64
SCALE = HD ** -0.5
NQ = 1024
HL = 3            # heads per core
HWID = HL * HD    # 192 head dims per core
DC = D // 128     # 6 contraction chunks

f16 = mybir.dt.float16
f32 = mybir.dt.float32

_programs = {}


def _build(SP: int):
    NCH = SP // 128
    nc = bacc.Bacc("TRN2", target_bir_lowering=False, debug=False, num_devices=8)

    qT = nc.dram_tensor("qT", [D, NQ], f16, kind="ExternalInput")
    kT = nc.dram_tensor("kT", [D, SP], f16, kind="ExternalInput")
    vT = nc.dram_tensor("vT", [D, SP], f16, kind="ExternalInput")
    mv = nc.dram_tensor("mv", [SP], f16, kind="ExternalInput")
    wqT = nc.dram_tensor("wqT", [D, HWID], f16, kind="ExternalInput")
    wkT = nc.dram_tensor("wkT", [D, HWID], f16, kind="ExternalInput")
    wvT = nc.dram_tensor("wvT", [D, HWID], f16, kind="ExternalInput")
    woT = nc.dram_tensor("woT", [HWID, D], f16, kind="ExternalInput")
    out = nc.dram_tensor("out", [NQ, D], f32, kind="ExternalOutput")

    EXPF = mybir.ActivationFunctionType.Exp
    qT_r = qT.ap().rearrange("(c p) n -> p c n", p=128)
    kT_r = kT.ap().rearrange("(c p) n -> p c n", p=128)
    vT_r = vT.ap().rearrange("(c p) n -> p c n", p=128)

    with tile.TileContext(nc) as tc:
        with (
            tc.tile_pool(name="const", bufs=1) as cpool,
            tc.tile_pool(name="work", bufs=2) as wpool,
            tc.tile_pool(name="expp", bufs=14) as epool,
            tc.tile_pool(name="ps", bufs=2, space="PSUM") as pspool,
            tc.tile_pool(name="psa", bufs=2, space="PSUM") as psapool,
        ):
            # ---- input DMAs, chunked + ordered so compute starts early
            wq_in = cpool.tile([128, DC, HWID], f16)
            nc.sync.dma_start(wq_in[:], wqT.ap().rearrange("(c p) n -> p c n", p=128))
            wk_in = cpool.tile([128, DC, HWID], f16)
            nc.sync.dma_start(wk_in[:], wkT.ap().rearrange("(c p) n -> p c n", p=128))
            SPA = min(1024, SP)          # first column block of kT/vT
            qT_in = cpool.tile([128, DC, NQ], f16)
            for d in range(DC):
                nc.sync.dma_start(qT_in[:, d, :], qT_r[:, d, :])
            kT_in = cpool.tile([128, DC, SP], f16)
            for d in range(DC):
                nc.sync.dma_start(kT_in[:, d, 0:SPA], kT_r[:, d, 0:SPA])
            wv_in = cpool.tile([128, DC, HWID], f16)
            nc.sync.dma_start(wv_in[:], wvT.ap().rearrange("(c p) n -> p c n", p=128))
            vT_in = cpool.tile([128, DC, SP], f16)
            for d in range(DC):
                nc.sync.dma_start(vT_in[:, d, 0:SPA], vT_r[:, d, 0:SPA])
            if SP > SPA:
                for d in range(DC):
                    nc.sync.dma_start(kT_in[:, d, SPA:SP], kT_r[:, d, SPA:SP])
                for d in range(DC):
                    nc.sync.dma_start(vT_in[:, d, SPA:SP], vT_r[:, d, SPA:SP])
            wo_in = cpool.tile([128, 2, D], f16)
            nc.sync.dma_start(wo_in[:, 0, :], woT[0:128, :])
            nc.sync.dma_start(wo_in[0:64, 1, :], woT[128:HWID, :])
            msk = cpool.tile([128, NCH], f16)
            nc.sync.dma_start(msk[:], mv.ap().rearrange("(c p) -> p c", p=128))

            q0 = cpool.tile([128, NQ], f16)
            q1 = cpool.tile([64, NQ], f16)
            k0 = cpool.tile([128, SP], f16)
            k1 = cpool.tile([64, SP], f16)
            vaug = cpool.tile([128, HL * NCH * 65], f16)
            vaug_r = vaug[:].rearrange("p (h j e) -> p h j e", h=HL, j=NCH)
            a0 = cpool.tile([128, NQ], f16)
            a1 = cpool.tile([64, NQ], f16)

            def proj_qk(w_in, src, dst, mt, sg, sw):
                mw = 128 if mt == 0 else 64
                ps = pspool.tile([mw, sw], f32, tag="ps")
                for d in range(DC):
                    for nf in range(0, sw, 512):
                        wf = min(512, sw - nf)
                        nc.tensor.matmul(
                            ps[:, nf:nf + wf],
                            w_in[:, d, mt * 128:mt * 128 + mw],
                            src[:, d, sg + nf:sg + nf + wf],
                            start=(d == 0), stop=(d == DC - 1),
                        )
                nc.vector.tensor_copy(dst[:, sg:sg + sw], ps[:])

            def proj_v(j):
                ps = pspool.tile([128, HWID], f32, tag="ps")
                for d in range(DC):
                    nc.tensor.matmul(
                        ps[:], vT_in[:, d, j * 128:(j + 1) * 128], wv_in[:, d, :],
                        start=(d == 0), stop=(d == DC - 1),
                    )
                nc.vector.tensor_copy(
                    vaug_r[:, :, j, 0:64], ps[:].rearrange("p (h e) -> p h e", h=HL)
                )

            def wo_mms(po, nt, kk, start, stop):
                asrc, kw = ((a0, 128), (a1, 64))[kk]
                for nf in range(0, D, 512):
                    wf = min(512, D - nf)
                    nc.tensor.matmul(
                        po[:, nf:nf + wf],
                        asrc[:, nt * 128:(nt + 1) * 128],
                        wo_in[0:kw, kk, nf:nf + wf],
                        start=start, stop=stop,
                    )

            LNF = mybir.ActivationFunctionType.Ln

            def normalize(at, adst):
                # 1/Z = exp(-ln Z), on ScalarE (keeps the DVE queue clear;
                # DVE's iterative reciprocal on [1, N] costs ~6.5us)
                lz = wpool.tile([1, NQ], f32, tag="lz")
                nc.scalar.activation(lz[:], at[64:65, :], LNF)
                rz = wpool.tile([1, NQ], f32, tag="rz")
                nc.scalar.activation(rz[:], lz[:], EXPF, scale=-1.0)
                rzb = wpool.tile([64, NQ], f32, tag="rzb")
                nc.gpsimd.partition_broadcast(rzb[:], rz[:])
                nc.vector.tensor_mul(adst, at[0:64, :], rzb[:])

            # mask column of vaug (depends only on msk DMA)
            nc.vector.tensor_copy(
                vaug_r[:, :, :, 64],
                msk[:].rearrange("p (u j) -> p u j", u=1).broadcast_to([128, HL, NCH]),
            )

            # ---- prologue: all projections, dense PE stream (DMA-paced)
            for sg in range(0, NQ, 1024):
                proj_qk(wq_in, qT_in, q0, 0, sg, min(1024, NQ - sg))
            for sg in range(0, SPA, 1024):
                proj_qk(wk_in, kT_in, k0, 0, sg, min(1024, SPA - sg))
            for j in range(SPA // 128):
                proj_v(j)
            for sg in range(0, NQ, 1024):
                proj_qk(wq_in, qT_in, q1, 1, sg, min(1024, NQ - sg))
            for sg in range(SPA, SP, 1024):
                proj_qk(wk_in, kT_in, k0, 0, sg, min(1024, SP - sg))
            for j in range(SPA // 128, NCH):
                proj_v(j)
            for sg in range(0, SP, 1024):
                proj_qk(wk_in, kT_in, k1, 1, sg, min(1024, SP - sg))

            # ---- fused h0+h1 attention (scores row-packed: K=64 pair at
            # base partitions 0/64 runs concurrently in the PE array)
            at0 = psapool.tile([65, NQ], f32, tag="at")
            at1 = psapool.tile([65, NQ], f32, tag="at")
            prev = None
            for j in range(NCH):
                sc0 = pspool.tile([128, NQ], f32, tag="ps")
                sc1 = pspool.tile([128, NQ], f32, tag="ps")
                for nf in range(0, NQ, 512):
                    nc.tensor.matmul(
                        sc0[:, nf:nf + 512], k0[0:64, j * 128:(j + 1) * 128],
                        q0[0:64, nf:nf + 512], start=True, stop=True,
                    )
                    nc.tensor.matmul(
                        sc1[:, nf:nf + 512], k0[64:128, j * 128:(j + 1) * 128],
                        q0[64:128, nf:nf + 512], start=True, stop=True,
                    )
                ex0 = epool.tile([128, NQ], f16, tag="ex")
                nc.scalar.activation(ex0[:], sc0[:], EXPF, scale=SCALE)
                ex1 = epool.tile([128, NQ], f16, tag="ex")
                nc.scalar.activation(ex1[:], sc1[:], EXPF, scale=SCALE)
                if prev is not None:
                    pj, pex0, pex1 = prev
                    for nf in range(0, NQ, 512):
                        nc.tensor.matmul(
                            at0[:, nf:nf + 512],
                            vaug[:, (0 * NCH + pj) * 65:(0 * NCH + pj) * 65 + 65],
                            pex0[:, nf:nf + 512], start=(pj == 0), stop=False,
                        )
                        nc.tensor.matmul(
                            at1[:, nf:nf + 512],
                            vaug[:, (1 * NCH + pj) * 65:(1 * NCH + pj) * 65 + 65],
                            pex1[:, nf:nf + 512], start=(pj == 0), stop=False,
                        )
                prev = (j, ex0, ex1)
            pj, pex0, pex1 = prev
            for nf in range(0, NQ, 512):
                nc.tensor.matmul(
                    at0[:, nf:nf + 512],
                    vaug[:, (0 * NCH + pj) * 65:(0 * NCH + pj) * 65 + 65],
                    pex0[:, nf:nf + 512], start=(pj == 0), stop=True,
                )
                nc.tensor.matmul(
                    at1[:, nf:nf + 512],
                    vaug[:, (1 * NCH + pj) * 65:(1 * NCH + pj) * 65 + 65],
                    pex1[:, nf:nf + 512], start=(pj == 0), stop=True,
                )
            normalize(at0, a0[0:64, :])
            normalize(at1, a0[64:128, :])

            # ---- h2 attention, with Wo kk=0 accumulation as PE filler
            at2 = psapool.tile([65, NQ], f32, tag="at")
            ob_a = cpool.tile([128, NQ // 128, D], f32)   # staged a0 @ WoT[0:128]
            prev = None
            for j in range(NCH):
                if j % 2 == 0 and j // 2 < NQ // 128:
                    nt = j // 2
                    po = psapool.tile([128, D], f32, tag="at")
                    wo_mms(po, nt, 0, True, True)
                    nc.vector.tensor_copy(ob_a[:, nt, :], po[:])
                sc = pspool.tile([128, NQ], f32, tag="ps")
                for nf in range(0, NQ, 512):
                    nc.tensor.matmul(
                        sc[:, nf:nf + 512], k1[:, j * 128:(j + 1) * 128],
                        q1[:, nf:nf + 512], start=True, stop=True,
                    )
                ex = epool.tile([128, NQ], f16, tag="ex")
                nc.scalar.activation(ex[:], sc[:], EXPF, scale=SCALE)
                if prev is not None:
                    pj, pex = prev
                    for nf in range(0, NQ, 512):
                        nc.tensor.matmul(
                            at2[:, nf:nf + 512],
                            vaug[:, (2 * NCH + pj) * 65:(2 * NCH + pj) * 65 + 65],
                            pex[:, nf:nf + 512], start=(pj == 0), stop=False,
                        )
                prev = (j, ex)
            pj, pex = prev
            for nf in range(0, NQ, 512):
                nc.tensor.matmul(
                    at2[:, nf:nf + 512],
                    vaug[:, (2 * NCH + pj) * 65:(2 * NCH + pj) * 65 + 65],
                    pex[:, nf:nf + 512], start=(pj == 0), stop=True,
                )
            normalize(at2, a1[:, :])

            # ---- finish Wo: kk=1 into PSUM, add staged kk=0 part, DMA out
            for nt in range(NQ // 128):
                po = pspool.tile([128, D], f32, tag="ps")
                wo_mms(po, nt, 1, True, True)
                ob = wpool.tile([128, D], f32, tag="ob")
                nc.vector.tensor_add(ob[:], po[:], ob_a[:, nt, :])
                nc.sync.dma_start(out[nt * 128:(nt + 1) * 128, :], ob[:])
    nc.compile()
    return nc


def _get_program(SP: int):
    if SP not in _programs:
        _programs[SP] = _build(SP)
    return _programs[SP]


def kernel(query, key, value, mask, Wq, Wk, Wv, Wo, bo):
    query = np.asarray(query, np.float32)
    key = np.asarray(key, np.float32)
    value = np.asarray(value, np.float32)
    mask = np.asarray(mask, np.float32)
    Wq = np.asarray(Wq, np.float32)
    Wk = np.asarray(Wk, np.float32)
    Wv = np.asarray(Wv, np.float32)
    Wo = np.asarray(Wo, np.float32)
    bo = np.asarray(bo, np.float32)

    B, N, _ = query.shape
    idxs = [np.nonzero(mask[b] > 0.5)[0] for b in range(B)]
    se_max = max(len(i) for i in idxs)
    SP = max(((se_max + 127) // 128) * 128, 128)
    nc = _get_program(SP)

    in_maps = []
    for c in range(8):
        b, g = c // 4, c % 4
        hs = g * HWID
        idx = idxs[b]
        ne = len(idx)
        kTc = np.zeros((D, SP), np.float16)
        kTc[:, :ne] = key[b].T[:, idx].astype(np.float16)
        vTc = np.zeros((D, SP), np.float16)
        vTc[:, :ne] = value[b].T[:, idx].astype(np.float16)
        mvec = np.zeros((SP,), np.float16)
        mvec[:ne] = 1.0
        in_maps.append({
            "qT": np.ascontiguousarray(query[b].T.astype(np.float16)),
            "kT": kTc,
            "vT": vTc,
            "mv": mvec,
            "wqT": np.ascontiguousarray(Wq[hs:hs + HWID, :].T.astype(np.float16)),
            "wkT": np.ascontiguousarray(Wk[hs:hs + HWID, :].T.astype(np.float16)),
            "wvT": np.ascontiguousarray(Wv[hs:hs + HWID, :].T.astype(np.float16)),
            "woT": np.ascontiguousarray(Wo[:, hs:hs + HWID].T.astype(np.float16)),
        })

    res = run_bass_kernel_spmd(nc, in_maps, list(range(8))).results
    out = np.zeros((B, N, D), np.float32)
    for b in range(B):
        out[b] = res[4 * b]["out"] + res[4 * b + 1]["out"] \
            + res[4 * b + 2]["out"] + res[4 * b + 3]["out"] + bo
    return out



# revision 13
# speedup vs baseline: 1.2312x; 1.2312x over previous
"""Cross-attention kernel for 8 Trainium2 NeuronCores.

Sharding: core c => batch b = c//4, head-group g = c%4 (3 of 12 heads, 192 dims).
Each core projects q/k/v for its heads, does softmax attention, and computes a
partial output projection (row-split Wo); host sums the 4 partials per batch.

Schedule (v2):
  - fill: packed-weight DMA + qT halves; q projections while kT/vT stream in.
  - pass1: per-group k projections + per-chunk v projections interleaved with
    h2 scores/exp/attn, so the Activation exp stream starts early and the PE
    stream stays dense while DMA feeds the rest of kT/vT.
  - pass2: h0+h1 scores/exp/attn (Activation-bound, PE has slack); h2 is
    normalized here on DVE+Pool, off the critical path.
  - tail: softmax denominators via DVE reciprocal_approx_fast (no Ln/Exp
    act-table loads), Pool partition-broadcast, DVE muls; Wo accumulates both
    contraction halves in PSUM (no SBUF staging / extra add).
  - mask compaction on host: only mask==1 key/value positions are shipped,
    zero-padded to a multiple of 128; padded rows have zero v and zero
    ones-column so they contribute 0 to numerator and denominator Z.
  - softmax without max-subtraction (scores*scale ~ N(0,1)); Z comes from a
    ones-column appended to v, so only the 64xN attention output is divided.
"""

import numpy as np

import concourse.bass as bass
import concourse.mybir as mybir
import concourse.tile as tile
from concourse import bacc
from concourse.bass_utils import run_bass_kernel_spmd

H = 12
D = 768
HD = 64
SCALE = HD ** -0.5
NQ = 1024
HL = 3            # heads per core
HWID = HL * HD    # 192 head dims per core
DC = D // 128     # 6 contraction chunks

f16 = mybir.dt.float16
f32 = mybir.dt.float32

_programs = {}
DEBUG = False


def _build(SP: int):
    NCH = SP // 128
    nc = bacc.Bacc("TRN2", target_bir_lowering=False, debug=False, num_devices=8)

    qT = nc.dram_tensor("qT", [D, NQ], f16, kind="ExternalInput")
    kT = nc.dram_tensor("kT", [D, SP], f16, kind="ExternalInput")
    vT = nc.dram_tensor("vT", [D, SP], f16, kind="ExternalInput")
    mv = nc.dram_tensor("mv", [SP], f16, kind="ExternalInput")
    wqkv = nc.dram_tensor("wqkv", [D, 3 * HWID], f16, kind="ExternalInput")
    wop = nc.dram_tensor("wop", [128, 1536], f16, kind="ExternalInput")
    out = nc.dram_tensor("out", [NQ, D], f16, kind="ExternalOutput")
    if DEBUG:
        dq0 = nc.dram_tensor("dq0", [128, NQ], f16, kind="ExternalOutput")
        dq1 = nc.dram_tensor("dq1", [64, NQ], f16, kind="ExternalOutput")
        dk0 = nc.dram_tensor("dk0", [128, SP], f16, kind="ExternalOutput")
        dk1 = nc.dram_tensor("dk1", [64, SP], f16, kind="ExternalOutput")
        dva = nc.dram_tensor("dva", [128, HL * (SP // 128) * 65], f16, kind="ExternalOutput")
        dat2 = nc.dram_tensor("dat2", [65, NQ], f32, kind="ExternalOutput")
        da0 = nc.dram_tensor("da0", [128, NQ], f16, kind="ExternalOutput")
        da1 = nc.dram_tensor("da1", [64, NQ], f16, kind="ExternalOutput")

    EXPF = mybir.ActivationFunctionType.Exp
    qT_r = qT.ap().rearrange("(c p) n -> p c n", p=128)
    kT_r = kT.ap().rearrange("(c p) n -> p c n", p=128)
    vT_r = vT.ap().rearrange("(c p) n -> p c n", p=128)
    wqkv_r = wqkv.ap().rearrange("(c p) n -> p c n", p=128)

    groups = [(j0, min(j0 + 4, NCH)) for j0 in range(0, NCH, 4)]

    with tile.TileContext(nc) as tc:
        with (
            tc.tile_pool(name="const", bufs=1) as cpool,
            tc.tile_pool(name="work", bufs=2) as wpool,
            tc.tile_pool(name="expp", bufs=6) as epool,
        ):
            # ---- input DMAs: critical slices (q1 weights, qT first half,
            # h2 k weights, first kT group) first, then the bulk streams
            w_in = cpool.tile([128, DC, 3 * HWID], f16)
            nc.sync.dma_start(w_in[:, :, 128:192], wqkv_r[:, :, 128:192])
            qT_in = cpool.tile([128, DC, NQ], f16)
            nc.sync.dma_start(qT_in[:, :, 0:512], qT_r[:, :, 0:512])
            nc.sync.dma_start(w_in[:, :, 320:384], wqkv_r[:, :, 320:384])
            kT_in = cpool.tile([128, DC, SP], f16)
            vT_in = cpool.tile([128, DC, SP], f16)
            j0, j1 = groups[0]
            nc.sync.dma_start(kT_in[:, :, j0 * 128:j1 * 128],
                              kT_r[:, :, j0 * 128:j1 * 128])
            nc.sync.dma_start(qT_in[:, :, 512:1024], qT_r[:, :, 512:1024])
            nc.sync.dma_start(w_in[:, :, 384:576], wqkv_r[:, :, 384:576])
            nc.sync.dma_start(vT_in[:, :, j0 * 128:j1 * 128],
                              vT_r[:, :, j0 * 128:j1 * 128])
            msk = cpool.tile([128, NCH], f16)
            nc.sync.dma_start(msk[:], mv.ap().rearrange("(c p) -> p c", p=128))
            nc.sync.dma_start(w_in[:, :, 0:128], wqkv_r[:, :, 0:128])
            nc.sync.dma_start(w_in[:, :, 192:320], wqkv_r[:, :, 192:320])
            for j0, j1 in groups[1:]:
                nc.sync.dma_start(kT_in[:, :, j0 * 128:j1 * 128],
                                  kT_r[:, :, j0 * 128:j1 * 128])
                nc.sync.dma_start(vT_in[:, :, j0 * 128:j1 * 128],
                                  vT_r[:, :, j0 * 128:j1 * 128])
            wo_in = cpool.tile([128, 1536], f16)
            nc.sync.dma_start(wo_in[:], wop.ap())

            # ---- SBUF holders
            q0 = cpool.tile([128, NQ], f16)
            q1 = cpool.tile([64, NQ], f16)
            k0 = cpool.tile([128, SP], f16)
            k1 = cpool.tile([64, SP], f16)
            vaug = cpool.tile([128, HL * NCH * 65], f16)
            vaug_r = vaug[:].rearrange("p (h j e) -> p h j e", h=HL, j=NCH)
            a0 = cpool.tile([128, NQ], f16)
            a1 = cpool.tile([64, NQ], f16)
            at2_sb = cpool.tile([65, NQ], f32)

            # ones column of vaug (gated by mask; zero for padded rows)
            nc.vector.tensor_copy(
                vaug_r[:, :, :, 64],
                msk[:].rearrange("p (u j) -> p u j", u=1).broadcast_to([128, HL, NCH]),
            )

            # softmax denominator: dst = num * (1/z) without touching the
            # Activation engine (DVE fast reciprocal + Pool broadcast).
            def normalize(z_ap, num_ap, dst_ap):
                # custom-DVE ops can't read partition-shifted APs on hw:
                # stage the Z row to partition 0 via an Activation copy first
                zrow = wpool.tile([1, 512], f32, tag="zrow")
                nc.scalar.copy(zrow[:], z_ap)
                rz = wpool.tile([1, 512], f32, tag="rz")
                nc.vector.reciprocal_approx_fast(rz[:], zrow[:])
                rzb = wpool.tile([64, 512], f32, tag="rzb")
                nc.gpsimd.partition_broadcast(rzb[:], rz[:])
                nc.vector.tensor_mul(dst_ap, num_ap, rzb[:])

            # =========== pool A: fill + pass1 (q/k/v proj + h2) ===========
            with tc.tile_pool(name="psA", bufs=1, space="PSUM") as pA:
                def qproj(dst, mt, mw, nf):
                    ps = pA.tile([128, NQ], f32, tag="qp")
                    for d in range(DC):
                        nc.tensor.matmul(
                            ps[0:mw, nf:nf + 512],
                            w_in[:, d, mt * 128:mt * 128 + mw],
                            qT_in[:, d, nf:nf + 512],
                            start=(d == 0), stop=(d == DC - 1),
                        )
                    nc.vector.tensor_copy(dst[:, nf:nf + 512], ps[0:mw, nf:nf + 512])

                qproj(q1, 1, 64, 0)           # h2 queries, first half

                at2a = pA.tile([65, 512], f32, tag="at2a")
                at2b = pA.tile([65, 512], f32, tag="at2b")

                prev = None
                for gi, (j0, j1) in enumerate(groups):
                    gw = (j1 - j0) * 128
                    # h2 k rows first (pass1 needs them); pair rows after
                    kp2 = pA.tile([128, 512], f32, tag="kp")
                    for d in range(DC):
                        nc.tensor.matmul(
                            kp2[0:64, 0:gw], w_in[:, d, HWID + 128:HWID + 192],
                            kT_in[:, d, j0 * 128:j1 * 128],
                            start=(d == 0), stop=(d == DC - 1),
                        )
                    nc.vector.tensor_copy(k1[:, j0 * 128:j1 * 128], kp2[0:64, 0:gw])
                    if gi == 0:
                        qproj(q1, 1, 64, 512)
                    kp = pA.tile([128, 512], f32, tag="kp")
                    for d in range(DC):
                        nc.tensor.matmul(
                            kp[:, 0:gw], w_in[:, d, HWID:HWID + 128],
                            kT_in[:, d, j0 * 128:j1 * 128],
                            start=(d == 0), stop=(d == DC - 1),
                        )
                    nc.vector.tensor_copy(k0[:, j0 * 128:j1 * 128], kp[:, 0:gw])

                    for j in range(j0, j1):
                        # v projection chunk j (all heads)
                        vp = pA.tile([128, HWID], f32, tag="vp")
                        for d in range(DC):
                            nc.tensor.matmul(
                                vp[:], vT_in[:, d, j * 128:(j + 1) * 128],
                                w_in[:, d, 2 * HWID:3 * HWID],
                                start=(d == 0), stop=(d == DC - 1),
                            )
                        nc.vector.tensor_copy(
                            vaug_r[:, :, j, 0:64],
                            vp[:].rearrange("p (h e) -> p h e", h=HL),
                        )
                        # h2 scores chunk j
                        sc = pA.tile([128, NQ], f32, tag="sc")
                        for nf in (0, 512):
                            nc.tensor.matmul(
                                sc[:, nf:nf + 512], k1[:, j * 128:(j + 1) * 128],
                                q1[:, nf:nf + 512], start=True, stop=True,
                            )
                        ex = epool.tile([128, NQ], f16, tag="ex")
                        nc.scalar.activation(ex[:], sc[:], EXPF, scale=SCALE)
                        # q0 projection squeezed in as PE filler, spread so
                        # the Activation stream keeps getting scores
                        if gi == 1 and j == j0:
                            qproj(q0, 0, 128, 0)
                        if gi == 2 and j == j0:
                            qproj(q0, 0, 128, 512)
                        if prev is not None:
                            pj, pex = prev
                            for nf, att in ((0, at2a), (512, at2b)):
                                nc.tensor.matmul(
                                    att[:, 0:512],
                                    vaug[:, (2 * NCH + pj) * 65:(2 * NCH + pj) * 65 + 65],
                                    pex[:, nf:nf + 512],
                                    start=(pj == 0), stop=False,
                                )
                        prev = (j, ex)
                pj, pex = prev
                for nf, att in ((0, at2a), (512, at2b)):
                    nc.tensor.matmul(
                        att[:, 0:512],
                        vaug[:, (2 * NCH + pj) * 65:(2 * NCH + pj) * 65 + 65],
                        pex[:, nf:nf + 512], start=(pj == 0), stop=True,
                    )
                # evacuate h2 accumulator so pool A can be released
                nc.vector.tensor_copy(at2_sb[:, 0:512], at2a[:])
                nc.vector.tensor_copy(at2_sb[:, 512:1024], at2b[:])

            # =========== pools B: pass2 (h0+h1) ===========
            with tc.tile_pool(name="psAt", bufs=1, space="PSUM") as pAt:
                at0a = pAt.tile([65, 512], f32, tag="at0a")
                at0b = pAt.tile([65, 512], f32, tag="at0b")
                at1a = pAt.tile([65, 512], f32, tag="at1a")
                at1b = pAt.tile([65, 512], f32, tag="at1b")
                with tc.tile_pool(name="psB", bufs=2, space="PSUM") as pB:
                    # h2 normalize: DVE+Pool only, overlaps pass2 compute
                    normalize(at2_sb[64:65, 0:512], at2_sb[0:64, 0:512],
                              a1[:, 0:512])
                    normalize(at2_sb[64:65, 512:1024], at2_sb[0:64, 512:1024],
                              a1[:, 512:1024])
                    prev = None
                    for j in range(NCH):
                        sc0 = pB.tile([128, NQ], f32, tag="sc2")
                        for nf in (0, 512):
                            nc.tensor.matmul(
                                sc0[:, nf:nf + 512], k0[0:64, j * 128:(j + 1) * 128],
                                q0[0:64, nf:nf + 512], start=True, stop=True,
                            )
                        ex0 = epool.tile([128, NQ], f16, tag="ex")
                        nc.scalar.activation(ex0[:], sc0[:], EXPF, scale=SCALE)
                        sc1 = pB.tile([128, NQ], f32, tag="sc2")
                        for nf in (0, 512):
                            nc.tensor.matmul(
                                sc1[:, nf:nf + 512], k0[64:128, j * 128:(j + 1) * 128],
                                q0[64:128, nf:nf + 512], start=True, stop=True,
                            )
                        ex1 = epool.tile([128, NQ], f16, tag="ex")
                        nc.scalar.activation(ex1[:], sc1[:], EXPF, scale=SCALE)
                        if prev is not None:
                            pj, pex0, pex1 = prev
                            for nf, atx, aty in ((0, at0a, at1a), (512, at0b, at1b)):
                                nc.tensor.matmul(
                                    atx[:, 0:512],
                                    vaug[:, (0 * NCH + pj) * 65:(0 * NCH + pj) * 65 + 65],
                                    pex0[:, nf:nf + 512], start=(pj == 0), stop=False,
                                )
                                nc.tensor.matmul(
                                    aty[:, 0:512],
                                    vaug[:, (1 * NCH + pj) * 65:(1 * NCH + pj) * 65 + 65],
                                    pex1[:, nf:nf + 512], start=(pj == 0), stop=False,
                                )
                        prev = (j, ex0, ex1)
                    pj, pex0, pex1 = prev
                    for nf, atx, aty in ((0, at0a, at1a), (512, at0b, at1b)):
                        nc.tensor.matmul(
                            atx[:, 0:512],
                            vaug[:, (0 * NCH + pj) * 65:(0 * NCH + pj) * 65 + 65],
                            pex0[:, nf:nf + 512], start=(pj == 0), stop=True,
                        )
                        nc.tensor.matmul(
                            aty[:, 0:512],
                            vaug[:, (1 * NCH + pj) * 65:(1 * NCH + pj) * 65 + 65],
                            pex1[:, nf:nf + 512], start=(pj == 0), stop=True,
                        )

                # =========== tail: normalize h0/h1 + Wo + out ===========
                with tc.tile_pool(name="psC", bufs=2, space="PSUM") as pC:
                    def wo_nt(nt):
                        po = pC.tile([128, D], f32, tag="po")
                        for nf, wf in ((0, 512), (512, 256)):
                            nc.tensor.matmul(
                                po[:, nf:nf + wf],
                                a0[:, nt * 128:(nt + 1) * 128],
                                wo_in[:, nf:nf + wf],
                                start=True, stop=False,
                            )
                        for nf, wf in ((0, 512), (512, 256)):
                            nc.tensor.matmul(
                                po[:, nf:nf + wf],
                                a1[:, nt * 128:(nt + 1) * 128],
                                wo_in[0:64, 768 + nf:768 + nf + wf],
                                start=False, stop=True,
                            )
                        ob = wpool.tile([128, D], f16, tag="ob", bufs=4)
                        if nt % 2 == 0:
                            nc.vector.tensor_copy(ob[:], po[:])
                        else:
                            nc.scalar.copy(ob[:], po[:])
                        nc.sync.dma_start(out[nt * 128:(nt + 1) * 128, :], ob[:])

                    normalize(at0a[64:65, :], at0a[0:64, :], a0[0:64, 0:512])
                    normalize(at1a[64:65, :], at1a[0:64, :], a0[64:128, 0:512])
                    for nt in range(4):
                        wo_nt(nt)
                    normalize(at0b[64:65, :], at0b[0:64, :], a0[0:64, 512:1024])
                    normalize(at1b[64:65, :], at1b[0:64, :], a0[64:128, 512:1024])
                    for nt in range(4, 8):
                        wo_nt(nt)
                    if DEBUG:
                        nc.sync.dma_start(dq0.ap(), q0[:])
                        nc.sync.dma_start(dq1.ap(), q1[:])
                        nc.sync.dma_start(dk0.ap(), k0[:])
                        nc.sync.dma_start(dk1.ap(), k1[:])
                        nc.sync.dma_start(dva.ap(), vaug[:])
                        nc.sync.dma_start(dat2.ap(), at2_sb[:])
                        nc.sync.dma_start(da0.ap(), a0[:])
                        nc.sync.dma_start(da1.ap(), a1[:])
    nc.compile()
    return nc


def _get_program(SP: int):
    if SP not in _programs:
        _programs[SP] = _build(SP)
    return _programs[SP]


def prepare(query, key, value, mask, Wq, Wk, Wv, Wo, bo):
    """Host prep: returns (nc, in_maps, assemble) where assemble(results)
    builds the full (B, N, D) output."""
    query = np.asarray(query, np.float32)
    key = np.asarray(key, np.float32)
    value = np.asarray(value, np.float32)
    mask = np.asarray(mask, np.float32)
    Wq = np.asarray(Wq, np.float32)
    Wk = np.asarray(Wk, np.float32)
    Wv = np.asarray(Wv, np.float32)
    Wo = np.asarray(Wo, np.float32)
    bo = np.asarray(bo, np.float32)

    B, N, _ = query.shape
    idxs = [np.nonzero(mask[b] > 0.5)[0] for b in range(B)]
    se_max = max(len(i) for i in idxs)
    SP = max(((se_max + 127) // 128) * 128, 128)
    nc = _get_program(SP)

    in_maps = []
    for c in range(8):
        b, g = c // 4, c % 4
        hs = g * HWID
        idx = idxs[b]
        ne = len(idx)
        kTc = np.zeros((D, SP), np.float16)
        kTc[:, :ne] = key[b].T[:, idx].astype(np.float16)
        vTc = np.zeros((D, SP), np.float16)
        vTc[:, :ne] = value[b].T[:, idx].astype(np.float16)
        mvec = np.zeros((SP,), np.float16)
        mvec[:ne] = 1.0
        wqkv = np.concatenate([
            Wq[hs:hs + HWID, :].T, Wk[hs:hs + HWID, :].T, Wv[hs:hs + HWID, :].T,
        ], axis=1).astype(np.float16)
        woT = Wo[:, hs:hs + HWID].T.astype(np.float16)   # [192, 768]
        wop = np.zeros((128, 1536), np.float16)
        wop[:, 0:768] = woT[0:128]
        wop[0:64, 768:1536] = woT[128:192]
        in_maps.append({
            "qT": np.ascontiguousarray(query[b].T.astype(np.float16)),
            "kT": kTc,
            "vT": vTc,
            "mv": mvec,
            "wqkv": np.ascontiguousarray(wqkv),
            "wop": wop,
        })

    def assemble(res):
        out = np.zeros((B, N, D), np.float32)
        for b in range(B):
            out[b] = res[4 * b]["out"].astype(np.float32) \
                + res[4 * b + 1]["out"].astype(np.float32) \
                + res[4 * b + 2]["out"].astype(np.float32) \
                + res[4 * b + 3]["out"].astype(np.float32) + bo
        return out

    return nc, in_maps, assemble


def kernel(query, key, value, mask, Wq, Wk, Wv, Wo, bo):
    nc, in_maps, assemble = prepare(query, key, value, mask, Wq, Wk, Wv, Wo, bo)
    res = run_bass_kernel_spmd(nc, in_maps, list(range(8))).results
    return assemble(res)
